# revision 32
# baseline (speedup 1.0000x reference)
"""BallLoss Trainium2 kernel (8-core data-parallel SPMD).

loss = sum_{i,j} relu(d_i - d_ij),  d_ij = ||e_i - c_j||, d_i = d_{i,label_i}

Per-core formulation (rows sharded across 8 cores, centers replicated):
  - PE:  p[i,j] = c2_j - 2*e_i.c_j   via augmented matmul with
         lhsT = [e_i; 1]^T (stationary), rhs = [-2*c; c2]^T (moving), fp32r.
  - ACT: dist[i,j] = sqrt(p[i,j] + e2_i)  (bias=e2 per partition),
         accum_out gives sumd_i = sum_j d_ij for free.
  - DVE: tensor_scalar max(dist, d_i) with accum_out=add gives
         macc_i = sum_j max(d_i, d_ij).
  - row result: sum_j relu(d_i - d_ij) = macc_i - sumd_i
    (since max(a,b) = b + relu(a-b)).
  - d_i computed exactly in fp32 from gathered centers (indirect DMA):
    d2_i = sum_d (e_id - c_{lab_i,d})^2, d_i = sqrt(d2_i).

Host: shards inputs, provides e / e^T layouts (layout prep only), sums the
8 per-core scalar partials.
"""

from contextlib import ExitStack

import numpy as np

import concourse.bass as bass
import concourse.tile as tile
from concourse import bacc, mybir
from concourse.bass_utils import run_bass_kernel_spmd

F32 = mybir.dt.float32
F32R = mybir.dt.float32r
BF16 = mybir.dt.bfloat16
I32 = mybir.dt.int32
AF = mybir.ActivationFunctionType
OP = mybir.AluOpType
AX = mybir.AxisListType

N, C, D = 65536, 2048, 64
NCORES = 8
NS = N // NCORES  # 8192 rows per core
P = 128           # partitions
T = NS // P       # 64 row-tiles per core
FD = 512          # fp32 psum bank free dim
NB = C // FD      # 4 matmuls per row-tile
G = 8             # row-tiles per precompute group
NG = T // G       # 8 groups

# bf16 matmul inputs; c2 is carried as two bf16 rows (hi + lo) so its
# precision stays ~2^-17. K = D + 2.
MM_DT = BF16
KA = D + 2


def _body(tc, out, eT, enat, labT, cT, cnat, dbg=None):
    nc = tc.nc
    with ExitStack() as ctx:
        const = ctx.enter_context(tc.tile_pool(name="const", bufs=1))

        eTa = const.tile([KA, NS], MM_DT)   # [66, 8192] rows 0..63 = e^T, 64,65 = 1
        chat = const.tile([KA, C], MM_DT)   # [66, 2048] 0..63 = -2c^T, 64/65 = c2 hi/lo
        craw = const.tile([D, C], F32)      # raw c^T
        ensb = const.tile([P, T * D], F32)    # [128, 4096] e natural, tile-major
        clab = const.tile([P, T * D], F32)    # gathered centers per row
        scr = const.tile([P, T * D], F32)     # scratch
        csq = const.tile([D, C], F32)         # c^T squared
        labsb = const.tile([P, T], I32)
        ones = const.tile([P, 1], F32)
        e2 = const.tile([P, T], F32)
        d2 = const.tile([P, T], F32)
        dall = const.tile([P, T], F32)
        macc = const.tile([P, T], F32)
        rowtot = const.tile([P, 1], F32)
        outsb = const.tile([1, 1], F32)

        # labels first: the gpsimd gather stream is gated only on this DMA
        nc.sync.dma_start(labsb[:], labT)
        nc.vector.memset(ones[:], 1.0)

        # chat build, pipelined per 512-col bank chunk (separate DMA queues)
        with tc.tile_pool(name="c2p", bufs=1, space="PSUM") as c2pool:
            c2ps = c2pool.tile([1, C], F32)
            for k in range(NB):
                sl = slice(k * FD, (k + 1) * FD)
                nc.sync.dma_start(craw[:, sl], cT[:, sl])
                nc.vector.tensor_mul(csq[:, sl], craw[:, sl], craw[:, sl])
                nc.tensor.matmul(
                    c2ps[:, sl], lhsT=ones[0:D, :], rhs=csq[:, sl],
                    start=True, stop=True,
                )
                # rows 0..63 = -2 * c^T, cast to bf16 by the DVE write
                nc.vector.tensor_scalar_mul(chat[0:D, sl], craw[:, sl], -2.0)
            # hi/lo computed at partition 0 (DVE can't start at partition 65),
            # then DMA'd into chat rows 64/65 (DMA has no partition alignment)
            c2hi = const.tile([1, C], MM_DT)
            c2lo = const.tile([1, C], MM_DT)
            nc.vector.tensor_copy(c2hi[:], c2ps[:])
            nc.vector.tensor_sub(c2lo[:], c2ps[:], c2hi[:])
            nc.sync.dma_start(chat[D:D + 1, :], c2hi[:])
            nc.sync.dma_start(chat[D + 1:KA, :], c2lo[:])

        # fused per-group: loads + gathers + (e2, d2, dall) precompute followed
        # immediately by that group's main tiles, so the in-order engine
        # streams interleave (precompute of group g overlaps tiles of g-1).
        stage_pool = ctx.enter_context(tc.tile_pool(name="stage", bufs=2))
        dist_pool = ctx.enter_context(tc.tile_pool(name="dist", bufs=6))
        z_pool = ctx.enter_context(tc.tile_pool(name="z", bufs=4))
        mm_ctx = tc.tile_pool(name="mm", bufs=2, space="PSUM")
        mm_pool = mm_ctx.__enter__()
        from concourse.tile import add_dep_helper

        z_insts = []
        for g in range(NG):
            cs, ce = g * G * P, (g + 1) * G * P
            fs, fe = g * G * D, (g + 1) * G * D
            ts, te = g * G, (g + 1) * G
            # loads
            stage = stage_pool.tile([KA, G * P], F32)
            nc.sync.dma_start(stage[:], eT[:, cs:ce])
            nc.vector.tensor_copy(eTa[:, cs:ce], stage[:])
            nc.sync.dma_start(
                ensb[:, fs:fe].rearrange("p (t d) -> p t d", d=D),
                enat[cs:ce, :].rearrange("(t p) d -> p t d", p=P),
            )
            for t in range(ts, te):
                nc.gpsimd.indirect_dma_start(
                    out=clab[:, t * D:(t + 1) * D],
                    out_offset=None,
                    in_=cnat,
                    in_offset=bass.IndirectOffsetOnAxis(ap=labsb[:, t:t + 1], axis=0),
                )
            # per-row e2, d2, d
            nc.vector.tensor_mul(scr[:, fs:fe], ensb[:, fs:fe], ensb[:, fs:fe])
            nc.vector.tensor_reduce(
                e2[:, ts:te], scr[:, fs:fe].rearrange("p (t d) -> p t d", d=D),
                axis=AX.X, op=OP.add,
            )
            sub_i = nc.vector.tensor_sub(
                clab[:, fs:fe], ensb[:, fs:fe], clab[:, fs:fe]
            )
            if g >= 1:
                # keep the gather-gated d2 chain BEHIND the previous group's
                # main DVE ops in the scheduled stream (the scheduler's DMA
                # model thinks indirect gathers are cheap; at runtime they'd
                # stall the whole in-order DVE stream if hoisted early)
                add_dep_helper(sub_i.ins, z_insts[g * G - 3].ins, sync=False,
                               reason="hold d2 chain behind prior group")
            nc.vector.tensor_mul(scr[:, fs:fe], clab[:, fs:fe], clab[:, fs:fe])
            nc.vector.tensor_reduce(
                d2[:, ts:te], scr[:, fs:fe].rearrange("p (t d) -> p t d", d=D),
                axis=AX.X, op=OP.add,
            )
            nc.scalar.activation(dall[:, ts:te], d2[:, ts:te], AF.Sqrt)

            # main tiles of this group
            for t in range(ts, te):
                ps = mm_pool.tile([P, C], F32, name="ps")
                lhsT = eTa[:, t * P:(t + 1) * P]
                for k in range(NB):
                    nc.tensor.matmul(
                        ps[:, k * FD:(k + 1) * FD],
                        lhsT=lhsT,
                        rhs=chat[:, k * FD:(k + 1) * FD],
                        start=True, stop=True,
                    )
                dist = dist_pool.tile([P, C], BF16, name="dist")
                nc.scalar.activation(
                    dist[:], ps[:], AF.Sqrt,
                    bias=e2[:, t:t + 1], scale=1.0,
                )
                if dbg is not None and t == 0:
                    psd = const.tile([P, C], F32, name="psdbg")
                    nc.vector.tensor_copy(psd[:], ps[:])
                    nc.sync.dma_start(dbg["ps0"], psd[:])
                    nc.sync.dma_start(dbg["dist0"], dist[:])
                # z = min(dist, d_i); macc_t = sum_j z
                z = z_pool.tile([P, C], BF16, name="z")
                if t % 5 == 2:
                    # ACT-reduced tile: DVE min at 4x, ACT sums via accum_out
                    zi = nc.vector.tensor_scalar(
                        out=z[:], in0=dist[:],
                        scalar1=dall[:, t:t + 1], scalar2=None,
                        op0=OP.min,
                    )
                    nc.scalar.activation(
                        z[:], z[:], AF.Identity,
                        accum_out=macc[:, t:t + 1],
                    )
                else:
                    # DVE-reduced tile: fused min + add-accum (1x)
                    zi = nc.vector.tensor_scalar(
                        out=z[:], in0=dist[:],
                        scalar1=dall[:, t:t + 1], scalar2=None,
                        op0=OP.min, op1=OP.add,
                        accum_out=macc[:, t:t + 1],
                    )
                z_insts.append(zi)
                if dbg is not None and t == 0:
                    nc.sync.dma_start(dbg["mx0"], z[:])

        mm_ctx.__exit__(None, None, None)

        # rowrelu[p,t] = C * dall - sum_j min(d_ij, d_i)  -> reuse macc
        nc.vector.scalar_tensor_tensor(
            out=macc[:], in0=dall[:], scalar=float(C), in1=macc[:],
            op0=OP.mult, op1=OP.subtract,
        )

        if dbg is not None:
            for name, tl in [("e2", e2), ("d2", d2), ("dall", dall),
                             ("macc", macc)]:
                nc.sync.dma_start(dbg[name], tl[:])

        # loss_partial = sum_{p,t} macc
        nc.vector.tensor_reduce(rowtot[:], macc[:], axis=AX.X, op=OP.add)
        with tc.tile_pool(name="fin", bufs=1, space="PSUM") as finp:
            fin = finp.tile([1, 1], F32)
            nc.tensor.matmul(fin[:], lhsT=rowtot[:], rhs=ones[:], start=True, stop=True)
            nc.scalar.copy(outsb[:], fin[:])
        nc.sync.dma_start(out, outsb[:])


_NC_CACHE = {}


def build_nc(debug=False):
    if debug in _NC_CACHE:
        return _NC_CACHE[debug]
    nc = bacc.Bacc(
        "TRN2", target_bir_lowering=False, debug=False, enable_asserts=False
    )
    eT = nc.dram_tensor("eT", [KA, NS], F32, kind="ExternalInput").ap()
    enat = nc.dram_tensor("enat", [NS, D], F32, kind="ExternalInput").ap()
    labT = nc.dram_tensor("labT", [P, T], I32, kind="ExternalInput").ap()
    cT = nc.dram_tensor("cT", [D, C], F32, kind="ExternalInput").ap()
    cnat = nc.dram_tensor("cnat", [C, D], F32, kind="ExternalInput").ap()
    out = nc.dram_tensor("out", [1, 1], F32, kind="ExternalOutput").ap()
    dbg = None
    if debug:
        dbg = {
            "ps0": nc.dram_tensor("ps0", [P, C], F32, kind="ExternalOutput").ap(),
            "dist0": nc.dram_tensor("dist0", [P, C], mybir.dt.bfloat16,
                                    kind="ExternalOutput").ap(),
            "mx0": nc.dram_tensor("mx0", [P, C], mybir.dt.bfloat16,
                                  kind="ExternalOutput").ap(),
            "e2": nc.dram_tensor("e2o", [P, T], F32, kind="ExternalOutput").ap(),
            "d2": nc.dram_tensor("d2o", [P, T], F32, kind="ExternalOutput").ap(),
            "dall": nc.dram_tensor("dallo", [P, T], F32, kind="ExternalOutput").ap(),
            "macc": nc.dram_tensor("macco", [P, T], F32, kind="ExternalOutput").ap(),
        }
    with tile.TileContext(nc) as tc:
        _body(tc, out, eT, enat, labT, cT, cnat, dbg=dbg)
    nc.compile()
    _NC_CACHE[debug] = nc
    return nc


def make_in_maps(embeddings, centers, labels):
    e = np.ascontiguousarray(np.asarray(embeddings, dtype=np.float32))
    c = np.ascontiguousarray(np.asarray(centers, dtype=np.float32))
    lab = np.asarray(labels).astype(np.int32)
    assert e.shape == (N, D) and c.shape == (C, D) and lab.shape == (N,)
    cT = np.ascontiguousarray(c.T)
    in_maps = []
    for core in range(NCORES):
        es = e[core * NS:(core + 1) * NS]
        ls = lab[core * NS:(core + 1) * NS]
        eT66 = np.ones((KA, NS), np.float32)
        eT66[0:D] = es.T
        in_maps.append({
            "eT": eT66,
            "enat": np.ascontiguousarray(es),
            "labT": np.ascontiguousarray(ls.reshape(T, P).T),
            "cT": cT,
            "cnat": c,
        })
    return in_maps


def run(embeddings, centers, labels, **kw):
    nc = build_nc()
    in_maps = make_in_maps(embeddings, centers, labels)
    res = run_bass_kernel_spmd(nc, in_maps, core_ids=list(range(NCORES)), **kw)
    total = float(sum(float(r["out"][0, 0]) for r in res.results))
    return np.float32(total), res


def kernel(embeddings, centers, labels):
    val, _ = run(embeddings, centers, labels)
    return val


# revision 34
# speedup vs baseline: 1.0193x; 1.0193x over previous
"""BallLoss Trainium2 kernel (8-core data-parallel SPMD).

loss = sum_{i,j} relu(d_i - d_ij),  d_ij = ||e_i - c_j||, d_i = d_{i,label_i}

Per-core formulation (rows sharded across 8 cores, centers replicated):
  - PE:  p[i,j] = c2_j - 2*e_i.c_j   via augmented matmul with
         lhsT = [e_i; 1]^T (stationary), rhs = [-2*c; c2]^T (moving), fp32r.
  - ACT: dist[i,j] = sqrt(p[i,j] + e2_i)  (bias=e2 per partition),
         accum_out gives sumd_i = sum_j d_ij for free.
  - DVE: tensor_scalar max(dist, d_i) with accum_out=add gives
         macc_i = sum_j max(d_i, d_ij).
  - row result: sum_j relu(d_i - d_ij) = macc_i - sumd_i
    (since max(a,b) = b + relu(a-b)).
  - d_i computed exactly in fp32 from gathered centers (indirect DMA):
    d2_i = sum_d (e_id - c_{lab_i,d})^2, d_i = sqrt(d2_i).

Host: shards inputs, provides e / e^T layouts (layout prep only), sums the
8 per-core scalar partials.
"""

from contextlib import ExitStack

import numpy as np

import concourse.bass as bass
import concourse.tile as tile
from concourse import bacc, mybir
from concourse.bass_utils import run_bass_kernel_spmd

F32 = mybir.dt.float32
F32R = mybir.dt.float32r
BF16 = mybir.dt.bfloat16
I32 = mybir.dt.int32
AF = mybir.ActivationFunctionType
OP = mybir.AluOpType
AX = mybir.AxisListType

N, C, D = 65536, 2048, 64
NCORES = 8
NS = N // NCORES  # 8192 rows per core
P = 128           # partitions
T = NS // P       # 64 row-tiles per core
FD = 512          # fp32 psum bank free dim
NB = C // FD      # 4 matmuls per row-tile
G = 8             # row-tiles per precompute group
NG = T // G       # 8 groups

# bf16 matmul inputs; c2 is carried as two bf16 rows (hi + lo) so its
# precision stays ~2^-17. K = D + 2.
MM_DT = BF16
KA = D + 2


def _body(tc, out, eT, enat, labT, cT, cnat, dbg=None):
    nc = tc.nc
    with ExitStack() as ctx:
        const = ctx.enter_context(tc.tile_pool(name="const", bufs=1))

        eTa = const.tile([KA, NS], MM_DT)   # [66, 8192] rows 0..63 = e^T, 64,65 = 1
        chat = const.tile([KA, C], MM_DT)   # [66, 2048] 0..63 = -2c^T, 64/65 = c2 hi/lo
        craw = const.tile([D, C], F32)      # raw c^T
        ensb = const.tile([P, T * D], F32)    # [128, 4096] e natural, tile-major
        clab = const.tile([P, T * D], F32)    # gathered centers per row
        scr = const.tile([P, T * D], F32)     # scratch
        csq = const.tile([D, C], F32)         # c^T squared
        labsb = const.tile([P, T], I32)
        ones = const.tile([P, 1], F32)
        e2 = const.tile([P, T], F32)
        d2 = const.tile([P, T], F32)
        dall = const.tile([P, T], F32)
        macc = const.tile([P, T], F32)
        rowtot = const.tile([P, 1], F32)
        outsb = const.tile([1, 1], F32)

        # labels first: the gpsimd gather stream is gated only on this DMA
        nc.sync.dma_start(labsb[:], labT)
        nc.vector.memset(ones[:], 1.0)

        # chat build, pipelined per 512-col bank chunk (separate DMA queues)
        with tc.tile_pool(name="c2p", bufs=1, space="PSUM") as c2pool:
            c2ps = c2pool.tile([1, C], F32)
            for k in range(NB):
                sl = slice(k * FD, (k + 1) * FD)
                nc.sync.dma_start(craw[:, sl], cT[:, sl])
                nc.vector.tensor_mul(csq[:, sl], craw[:, sl], craw[:, sl])
                nc.tensor.matmul(
                    c2ps[:, sl], lhsT=ones[0:D, :], rhs=csq[:, sl],
                    start=True, stop=True,
                )
                # rows 0..63 = -2 * c^T, cast to bf16 by the DVE write
                nc.vector.tensor_scalar_mul(chat[0:D, sl], craw[:, sl], -2.0)
            # hi/lo computed at partition 0 (DVE can't start at partition 65),
            # then DMA'd into chat rows 64/65 (DMA has no partition alignment)
            c2hi = const.tile([1, C], MM_DT)
            c2lo = const.tile([1, C], MM_DT)
            nc.vector.tensor_copy(c2hi[:], c2ps[:])
            c2lo_i = nc.vector.tensor_sub(c2lo[:], c2ps[:], c2hi[:])
            nc.sync.dma_start(chat[D:D + 1, :], c2hi[:])
            nc.sync.dma_start(chat[D + 1:KA, :], c2lo[:])

        # fused per-group: loads + gathers + (e2, d2, dall) precompute followed
        # immediately by that group's main tiles, so the in-order engine
        # streams interleave (precompute of group g overlaps tiles of g-1).
        stage_pool = ctx.enter_context(tc.tile_pool(name="stage", bufs=2))
        dist_pool = ctx.enter_context(tc.tile_pool(name="dist", bufs=6))
        z_pool = ctx.enter_context(tc.tile_pool(name="z", bufs=4))
        mm_ctx = tc.tile_pool(name="mm", bufs=2, space="PSUM")
        mm_pool = mm_ctx.__enter__()
        from concourse.tile import add_dep_helper

        z_insts = []
        for g in range(NG):
            cs, ce = g * G * P, (g + 1) * G * P
            fs, fe = g * G * D, (g + 1) * G * D
            ts, te = g * G, (g + 1) * G
            # loads
            stage = stage_pool.tile([KA, G * P], F32)
            nc.sync.dma_start(stage[:], eT[:, cs:ce])
            nc.vector.tensor_copy(eTa[:, cs:ce], stage[:])
            nc.sync.dma_start(
                ensb[:, fs:fe].rearrange("p (t d) -> p t d", d=D),
                enat[cs:ce, :].rearrange("(t p) d -> p t d", p=P),
            )
            for t in range(ts, te):
                nc.gpsimd.indirect_dma_start(
                    out=clab[:, t * D:(t + 1) * D],
                    out_offset=None,
                    in_=cnat,
                    in_offset=bass.IndirectOffsetOnAxis(ap=labsb[:, t:t + 1], axis=0),
                )
            # per-row e2, d2, d
            nc.vector.tensor_mul(scr[:, fs:fe], ensb[:, fs:fe], ensb[:, fs:fe])
            nc.vector.tensor_reduce(
                e2[:, ts:te], scr[:, fs:fe].rearrange("p (t d) -> p t d", d=D),
                axis=AX.X, op=OP.add,
            )
            sub_i = nc.vector.tensor_sub(
                clab[:, fs:fe], ensb[:, fs:fe], clab[:, fs:fe]
            )
            if g >= 1:
                # keep the gather-gated d2 chain BEHIND the previous group's
                # main DVE ops in the scheduled stream (the scheduler's DMA
                # model thinks indirect gathers are cheap; at runtime they'd
                # stall the whole in-order DVE stream if hoisted early)
                add_dep_helper(sub_i.ins, z_insts[g * G - 3].ins, sync=False,
                               reason="hold d2 chain behind prior group")
            else:
                # ... and behind the chat build for group 0, so the first
                # main matmul isn't stuck behind the gather stall
                add_dep_helper(sub_i.ins, c2lo_i.ins, sync=False,
                               reason="hold g0 d2 chain behind chat build")
            nc.vector.tensor_mul(scr[:, fs:fe], clab[:, fs:fe], clab[:, fs:fe])
            nc.vector.tensor_reduce(
                d2[:, ts:te], scr[:, fs:fe].rearrange("p (t d) -> p t d", d=D),
                axis=AX.X, op=OP.add,
            )
            nc.scalar.activation(dall[:, ts:te], d2[:, ts:te], AF.Sqrt)

            # main tiles of this group
            for t in range(ts, te):
                ps = mm_pool.tile([P, C], F32, name="ps")
                lhsT = eTa[:, t * P:(t + 1) * P]
                for k in range(NB):
                    nc.tensor.matmul(
                        ps[:, k * FD:(k + 1) * FD],
                        lhsT=lhsT,
                        rhs=chat[:, k * FD:(k + 1) * FD],
                        start=True, stop=True,
                    )
                dist = dist_pool.tile([P, C], BF16, name="dist")
                nc.scalar.activation(
                    dist[:], ps[:], AF.Sqrt,
                    bias=e2[:, t:t + 1], scale=1.0,
                )
                if dbg is not None and t == 0:
                    psd = const.tile([P, C], F32, name="psdbg")
                    nc.vector.tensor_copy(psd[:], ps[:])
                    nc.sync.dma_start(dbg["ps0"], psd[:])
                    nc.sync.dma_start(dbg["dist0"], dist[:])
                # z = min(dist, d_i); macc_t = sum_j z
                z = z_pool.tile([P, C], BF16, name="z")
                if t % 5 == 2:
                    # ACT-reduced tile: DVE min at 4x, ACT sums via accum_out
                    zi = nc.vector.tensor_scalar(
                        out=z[:], in0=dist[:],
                        scalar1=dall[:, t:t + 1], scalar2=None,
                        op0=OP.min,
                    )
                    nc.scalar.activation(
                        z[:], z[:], AF.Identity,
                        accum_out=macc[:, t:t + 1],
                    )
                else:
                    # DVE-reduced tile: fused min + add-accum (1x)
                    zi = nc.vector.tensor_scalar(
                        out=z[:], in0=dist[:],
                        scalar1=dall[:, t:t + 1], scalar2=None,
                        op0=OP.min, op1=OP.add,
                        accum_out=macc[:, t:t + 1],
                    )
                z_insts.append(zi)
                if dbg is not None and t == 0:
                    nc.sync.dma_start(dbg["mx0"], z[:])

        mm_ctx.__exit__(None, None, None)

        # rowrelu[p,t] = C * dall - sum_j min(d_ij, d_i)  -> reuse macc
        nc.vector.scalar_tensor_tensor(
            out=macc[:], in0=dall[:], scalar=float(C), in1=macc[:],
            op0=OP.mult, op1=OP.subtract,
        )

        if dbg is not None:
            for name, tl in [("e2", e2), ("d2", d2), ("dall", dall),
                             ("macc", macc)]:
                nc.sync.dma_start(dbg[name], tl[:])

        # loss_partial = sum_{p,t} macc
        nc.vector.tensor_reduce(rowtot[:], macc[:], axis=AX.X, op=OP.add)
        with tc.tile_pool(name="fin", bufs=1, space="PSUM") as finp:
            fin = finp.tile([1, 1], F32)
            nc.tensor.matmul(fin[:], lhsT=rowtot[:], rhs=ones[:], start=True, stop=True)
            nc.scalar.copy(outsb[:], fin[:])
        nc.sync.dma_start(out, outsb[:])


_NC_CACHE = {}


def build_nc(debug=False):
    if debug in _NC_CACHE:
        return _NC_CACHE[debug]
    nc = bacc.Bacc(
        "TRN2", target_bir_lowering=False, debug=False, enable_asserts=False
    )
    eT = nc.dram_tensor("eT", [KA, NS], F32, kind="ExternalInput").ap()
    enat = nc.dram_tensor("enat", [NS, D], F32, kind="ExternalInput").ap()
    labT = nc.dram_tensor("labT", [P, T], I32, kind="ExternalInput").ap()
    cT = nc.dram_tensor("cT", [D, C], F32, kind="ExternalInput").ap()
    cnat = nc.dram_tensor("cnat", [C, D], F32, kind="ExternalInput").ap()
    out = nc.dram_tensor("out", [1, 1], F32, kind="ExternalOutput").ap()
    dbg = None
    if debug:
        dbg = {
            "ps0": nc.dram_tensor("ps0", [P, C], F32, kind="ExternalOutput").ap(),
            "dist0": nc.dram_tensor("dist0", [P, C], mybir.dt.bfloat16,
                                    kind="ExternalOutput").ap(),
            "mx0": nc.dram_tensor("mx0", [P, C], mybir.dt.bfloat16,
                                  kind="ExternalOutput").ap(),
            "e2": nc.dram_tensor("e2o", [P, T], F32, kind="ExternalOutput").ap(),
            "d2": nc.dram_tensor("d2o", [P, T], F32, kind="ExternalOutput").ap(),
            "dall": nc.dram_tensor("dallo", [P, T], F32, kind="ExternalOutput").ap(),
            "macc": nc.dram_tensor("macco", [P, T], F32, kind="ExternalOutput").ap(),
        }
    with tile.TileContext(nc) as tc:
        _body(tc, out, eT, enat, labT, cT, cnat, dbg=dbg)
    nc.compile()
    _NC_CACHE[debug] = nc
    return nc


def make_in_maps(embeddings, centers, labels):
    e = np.ascontiguousarray(np.asarray(embeddings, dtype=np.float32))
    c = np.ascontiguousarray(np.asarray(centers, dtype=np.float32))
    lab = np.asarray(labels).astype(np.int32)
    assert e.shape == (N, D) and c.shape == (C, D) and lab.shape == (N,)
    cT = np.ascontiguousarray(c.T)
    in_maps = []
    for core in range(NCORES):
        es = e[core * NS:(core + 1) * NS]
        ls = lab[core * NS:(core + 1) * NS]
        eT66 = np.ones((KA, NS), np.float32)
        eT66[0:D] = es.T
        in_maps.append({
            "eT": eT66,
            "enat": np.ascontiguousarray(es),
            "labT": np.ascontiguousarray(ls.reshape(T, P).T),
            "cT": cT,
            "cnat": c,
        })
    return in_maps


def run(embeddings, centers, labels, **kw):
    nc = build_nc()
    in_maps = make_in_maps(embeddings, centers, labels)
    res = run_bass_kernel_spmd(nc, in_maps, core_ids=list(range(NCORES)), **kw)
    total = float(sum(float(r["out"][0, 0]) for r in res.results))
    return np.float32(total), res


def kernel(embeddings, centers, labels):
    val, _ = run(embeddings, centers, labels)
    return val
